# revision 1
# baseline (speedup 1.0000x reference)
"""Block-causal self-attention on 8 Trainium2 NeuronCores.

Sharding: data-parallel over batch (B=8 -> one batch element per core).
Weights replicated. No collectives.

Per-core Bass program (fp32 storage, fp32r matmuls ~ full PE rate at N>=256):
  - inputs arrive pre-transposed on host: xT=[C,T], w*T=[C,C] (c-major)
  - qT,kT = W @ xT + b   (feature-on-partition layout [C,T])
  - v     = x @ WvT + bv (natural [T,C]) stored per-head with two ones columns
  - per (head, 256-query pair), transposed-scores flash attention:
      scores^T[key, query] = kT_tile.T @ qT  (K=d=64)
      + rank-17 matmul adds the block-causal fine mask as a 0/-8192 bias
        (16 frame-indicator rows x frame-step rows + constant row)
      ACT exp (scale=1/sqrt(d) fused; general mask path adds per-key bias)
      AV: psY[0:66] += v_aug.T @ p  (ones cols give softmax denominator rows)
    only lower-triangular key tiles are computed; diagonal processed first
  - normalize: r = 1/l (DVE), gpsimd partition_broadcast, DVE multiply -> yT
  - out = yT proj + bp (rank-1 bias matmul), ACT copy, DMA per 128-row tile
"""

import contextlib
import math

import numpy as np

import concourse.bass as bass
import concourse.mybir as mybir
import concourse.tile as tile
from concourse import bacc
from concourse.bass_utils import run_bass_kernel_spmd

F32 = mybir.dt.float32
F32R = mybir.dt.float32r
EXP = mybir.ActivationFunctionType.Exp
IDENT = mybir.ActivationFunctionType.Identity

B, T, C = 8, 1024, 512
H = 8
D = C // H          # 64
NF = 128            # frames
NA = 8              # animals per frame
NT = T // 128       # 8 query/key tiles of 128
NC4 = C // 128      # 4 feature tiles
NEG = -1e9


def build_attention(tc, out_ap, ins, general_mask):
    """Emit the per-core attention program into TileContext tc.

    ins: dict of input APs (DRAM).
    """
    nc = tc.nc
    xT, wqT, wkT, wvT, wpT = ins["xT"], ins["wqT"], ins["wkT"], ins["wvT"], ins["wpT"]
    bq_t, bk_t = ins["bq_t"], ins["bk_t"]
    bv_row, bp_row = ins["bv_row"], ins["bp_row"]
    ones_in = ins["ones_in"]
    kaug_in, qaug_in = ins["kaug"], ins["qaug"]
    kmask = ins.get("kmask")

    # ---------------- persistent SBUF tiles ----------------
    frees = []

    def t_sb(name, shape, dtype=F32R):
        tl, free = tc.tile(shape, dtype, name=name)
        frees.append(free)
        return tl

    xt = [t_sb(f"xt{i}", [128, T]) for i in range(NC4)]
    wq = [t_sb(f"wq{i}", [128, C]) for i in range(NC4)]
    wk = [t_sb(f"wk{i}", [128, C]) for i in range(NC4)]
    wv = [t_sb(f"wv{i}", [128, C]) for i in range(NC4)]
    wp = [t_sb(f"wp{i}", [128, C]) for i in range(NC4)]
    qT = [t_sb(f"qT{i}", [128, T]) for i in range(NC4)]
    kT = [t_sb(f"kT{i}", [128, T]) for i in range(NC4)]
    # v per t-tile: [128, head, 66]; v cols 0:64, ones cols 64:66 (even
    # stationary free dim for fp32r; row 64 of the AV output = softmax denom)
    vt = [t_sb(f"vt{i}", [128, H, 66]) for i in range(NT)]
    yT = [t_sb(f"yT{i}", [128, T]) for i in range(NC4)]
    ones_sb = t_sb("ones_sb", [128, 128])
    kaug_sb = t_sb("kaug_sb", [17, 128])
    qaug_sb = t_sb("qaug_sb", [17, 512])
    bq_sb = t_sb("bq_sb", [128, NC4], F32)
    bk_sb = t_sb("bk_sb", [128, NC4], F32)
    bv_sb = t_sb("bv_sb", [1, C])
    bp_sb = t_sb("bp_sb", [1, C])
    km_sb = t_sb("km_sb", [128, NT], F32) if general_mask else None

    # ---------------- input DMAs ----------------
    for i in range(NC4):
        nc.sync.dma_start(out=xt[i][:, 0:512],
                          in_=xT[i * 128:(i + 1) * 128, 0:512].bitcast(F32R))
    for i in range(NC4):
        nc.sync.dma_start(out=wv[i], in_=wvT[i * 128:(i + 1) * 128, :].bitcast(F32R))
    for i in range(NC4):
        nc.sync.dma_start(out=xt[i][:, 512:T],
                          in_=xT[i * 128:(i + 1) * 128, 512:T].bitcast(F32R))
    for i in range(NC4):
        nc.sync.dma_start(out=wq[i], in_=wqT[i * 128:(i + 1) * 128, :].bitcast(F32R))
        nc.sync.dma_start(out=wk[i], in_=wkT[i * 128:(i + 1) * 128, :].bitcast(F32R))
    nc.sync.dma_start(out=kaug_sb, in_=kaug_in.bitcast(F32R))
    nc.sync.dma_start(out=qaug_sb, in_=qaug_in.bitcast(F32R))
    nc.sync.dma_start(out=bq_sb, in_=bq_t)
    nc.sync.dma_start(out=bk_sb, in_=bk_t)
    nc.sync.dma_start(out=bv_sb, in_=bv_row.bitcast(F32R))
    nc.sync.dma_start(out=bp_sb, in_=bp_row.bitcast(F32R))
    for i in range(NC4):
        nc.sync.dma_start(out=wp[i], in_=wpT[i * 128:(i + 1) * 128, :].bitcast(F32R))
    if general_mask:
        nc.sync.dma_start(out=km_sb, in_=kmask)
    nc.sync.dma_start(out=ones_sb, in_=ones_in.bitcast(F32R))
    for i in range(NT):
        # ones cols (softmax denominator rows in the AV matmul)
        nc.sync.dma_start(out=vt[i][:, :, 64:66],
                          in_=ones_in[:, 0:16].rearrange("p (h o) -> p h o", h=H).bitcast(F32R))

    # ---------------- pools ----------------
    ctx = contextlib.ExitStack()
    with ctx:
        mm_pool = ctx.enter_context(tc.tile_pool(name="mm", bufs=4, space="PSUM"))
        py_pool = ctx.enter_context(tc.tile_pool(name="py", bufs=2, space="PSUM"))
        pe_pool = ctx.enter_context(tc.tile_pool(name="pe", bufs=12))
        rr_pool = ctx.enter_context(tc.tile_pool(name="rr", bufs=2))
        ob_pool = ctx.enter_context(tc.tile_pool(name="ob", bufs=4))

        # ---------------- phase 1+2 interleaved ----------------
        def emit_v():
            for tt in range(NT):
                psv = py_pool.tile([128, T], F32, tag="py", name=f"psv{tt}")[:, 0:512]
                for c in range(NC4):
                    nc.tensor.matmul(
                        psv,
                        xt[c][:, tt * 128:(tt + 1) * 128],
                        wv[c],
                        start=(c == 0), stop=False)
                nc.tensor.matmul(psv, ones_sb[0:1, 0:128],
                                 bv_sb, start=False, stop=True)
                psv3 = psv.rearrange("p (h d) -> p h d", h=H)
                nc.vector.tensor_copy(vt[tt][:, :, 0:64], psv3)


        def emit_qk(i):
            if True:
                for ch in range(2):
                    tsl = slice(ch * 512, ch * 512 + 512)
                    psq = mm_pool.tile([128, 512], F32, tag="mm", name=f"psq{i}{ch}")
                    for c in range(NC4):
                        nc.tensor.matmul(
                            psq,
                            wq[c][:, i * 128:(i + 1) * 128],
                            xt[c][:, tsl],
                            start=(c == 0), stop=(c == NC4 - 1))
                    nc.vector.tensor_scalar_add(qT[i][:, tsl], psq,
                                                bq_sb[:, i:i + 1])
                    psk = mm_pool.tile([128, 512], F32, tag="mm", name=f"psk{i}{ch}")
                    for c in range(NC4):
                        nc.tensor.matmul(
                            psk,
                            wk[c][:, i * 128:(i + 1) * 128],
                            xt[c][:, tsl],
                            start=(c == 0), stop=(c == NC4 - 1))
                    nc.vector.tensor_scalar_add(kT[i][:, tsl], psk, bk_sb[:, i:i + 1])


        def emit_head(h):
            ht, hr = h // 2, (h % 2) * 64
            lrow = 64
            ysl = slice(0, 64)
            avsl = slice(0, 66)
            scale = 1.0 / math.sqrt(D)
            psY = py_pool.tile([128, T], F32, tag="py", name=f"psY{h}")
            for p in range(4):
                cols = slice(p * 256, p * 256 + 256)
                for kk in [p] + list(range(p)):
                    diag = kk == p
                    psS = mm_pool.tile([128, 512], F32, tag="mm",
                                       name=f"psS{h}{p}{kk}")
                    for half in range(2):
                        ki = 2 * kk + half
                        hsl = slice(half * 256, half * 256 + 256)
                        if diag:  # block-causal mask bias (0 / -BIG) first:
                            # depends only on constant tiles, off the
                            # score->exp critical path
                            nc.tensor.matmul(psS[:, hsl], kaug_sb,
                                             qaug_sb[:, hsl],
                                             start=True, stop=False)
                        nc.tensor.matmul(
                            psS[:, hsl],
                            kT[ht][hr:hr + 64, ki * 128:(ki + 1) * 128],
                            qT[ht][hr:hr + 64, cols],
                            start=not diag, stop=True)
                    pexp = pe_pool.tile([128, 512], F32R, tag="pe",
                                        name=f"pexp{h}{p}{kk}")
                    if general_mask:
                        for half in range(2):
                            ki = 2 * kk + half
                            hsl = slice(half * 256, half * 256 + 256)
                            nc.scalar.activation(
                                out=pexp[:, hsl], in_=psS[:, hsl], func=EXP,
                                bias=km_sb[:, ki:ki + 1], scale=scale)
                    else:
                        nc.scalar.activation(out=pexp, in_=psS, func=EXP,
                                             scale=scale)
                    first_ki = 2 * p
                    last_ki = 2 * p - 1 if p > 0 else 2 * p + 1
                    for half in range(2):
                        ki = 2 * kk + half
                        nc.tensor.matmul(
                            psY[avsl, cols],
                            vt[ki][:, h, :],
                            pexp[:, half * 256:half * 256 + 256],
                            start=(ki == first_ki), stop=(ki == last_ki))
            # normalize: r = 1/l, broadcast over 64 partitions, multiply
            # (two 512-col halves so the output projection can start on the
            # first half while the second is still in flight)
            rrow = rr_pool.tile([1, T], F32, tag="rr", name=f"rrow{h}")
            rrep = rr_pool.tile([64, T], F32, tag="rrep", name=f"rrep{h}")
            for cc in range(2):
                csl = slice(cc * 512, cc * 512 + 512)
                nc.vector.reciprocal(rrow[:, csl], psY[lrow:lrow + 1, csl])
                nc.gpsimd.partition_broadcast(rrep[:, csl], rrow[:, csl])
                nc.vector.tensor_mul(yT[ht][hr:hr + 64, csl], psY[ysl, csl],
                                     rrep[:, csl])


        emit_v()
        for i in range(NC4):
            emit_qk(i)
        for h in range(H):
            emit_head(h)

        # ---------------- phase 3: output projection ----------------
        for tt in range(NT):
            pso = mm_pool.tile([128, 512], F32, tag="mm", name=f"pso{tt}")
            for c in range(NC4):
                nc.tensor.matmul(
                    pso,
                    yT[c][:, tt * 128:(tt + 1) * 128],
                    wp[c],
                    start=(c == 0), stop=False)
            nc.tensor.matmul(pso, ones_sb[0:1, 0:128],
                             bp_sb, start=False, stop=True)
            o_sb = ob_pool.tile([128, 512], F32, tag="ob", name=f"osb{tt}")
            nc.scalar.copy(o_sb, pso)
            nc.sync.dma_start(out=out_ap[tt * 128:(tt + 1) * 128, :], in_=o_sb)

    for f in reversed(frees):
        f()


# ---------------------------------------------------------------------------
# host side
# ---------------------------------------------------------------------------

BIG = 8192.0


def _aug_mask_tiles():
    """Rank-17 additive encoding of the diagonal block-causal mask.

    bias[tk, c] = BIG * (w[tk//8, c] - 1): 0 where allowed, -BIG where masked.
    kaug [17, 128]: rows f<16: BIG * [tk//8 == f]; row 16: ones.
    qaug [17, 512]: cols 0:256 for key tile ki=2p (fine | allowed),
                    cols 256:512 for ki=2p+1 (masked | fine); row 16: -BIG."""
    a = np.arange(128)
    f = np.arange(16)
    kaug = np.zeros((17, 128), np.float32)
    kaug[:16] = BIG * (a[None, :] // NA == f[:, None])
    kaug[16] = 1.0
    fine = (a[None, :] // NA >= f[:, None]).astype(np.float32)  # [16, 128]
    qaug = np.zeros((17, 512), np.float32)
    qaug[:16, 0:128] = fine
    qaug[:16, 128:256] = 1.0
    qaug[:16, 256:384] = 0.0
    qaug[:16, 384:512] = fine
    qaug[16] = -BIG
    return kaug, qaug


def make_host_inputs(x, mask, Wq, bq, Wk, bk, Wv, bv, Wp, bp):
    """Returns (per_core_inputs, general_mask)."""
    f32 = np.float32
    x = np.asarray(x, dtype=f32)
    mask = np.asarray(mask, dtype=f32)
    Wq, bq = np.asarray(Wq, dtype=f32), np.asarray(bq, dtype=f32)
    Wk, bk = np.asarray(Wk, dtype=f32), np.asarray(bk, dtype=f32)
    Wv, bv = np.asarray(Wv, dtype=f32), np.asarray(bv, dtype=f32)
    Wp, bp = np.asarray(Wp, dtype=f32), np.asarray(bp, dtype=f32)
    general_mask = not bool(np.all(mask == 1.0))
    shared = {
        "wqT": np.ascontiguousarray(Wq.T.astype(f32)),
        "wkT": np.ascontiguousarray(Wk.T.astype(f32)),
        "wvT": np.ascontiguousarray(Wv.T.astype(f32)),
        "wpT": np.ascontiguousarray(Wp.T.astype(f32)),
        "bq_t": np.ascontiguousarray(bq.astype(f32).reshape(NC4, 128).T),
        "bk_t": np.ascontiguousarray(bk.astype(f32).reshape(NC4, 128).T),
        "bv_row": bv.astype(f32).reshape(1, C).copy(),
        "bp_row": bp.astype(f32).reshape(1, C).copy(),
        "ones_in": np.ones((128, 128), np.float32),
    }
    shared["kaug"], shared["qaug"] = _aug_mask_tiles()
    per_core = []
    for b in range(B):
        d = dict(shared)
        d["xT"] = np.ascontiguousarray(x[b].astype(f32).T)
        if general_mask:
            km = np.where(mask[b] != 0, 0.0, NEG).astype(f32)
            d["kmask"] = np.ascontiguousarray(km.reshape(NT, 128).T)
        per_core.append(d)
    return per_core, general_mask


def build_program(general_mask=False):
    nc = bacc.Bacc("TRN2", target_bir_lowering=False, debug=False, num_devices=1)
    ins = {
        "xT": nc.dram_tensor("xT", [C, T], F32, kind="ExternalInput").ap(),
        "wqT": nc.dram_tensor("wqT", [C, C], F32, kind="ExternalInput").ap(),
        "wkT": nc.dram_tensor("wkT", [C, C], F32, kind="ExternalInput").ap(),
        "wvT": nc.dram_tensor("wvT", [C, C], F32, kind="ExternalInput").ap(),
        "wpT": nc.dram_tensor("wpT", [C, C], F32, kind="ExternalInput").ap(),
        "bq_t": nc.dram_tensor("bq_t", [128, NC4], F32, kind="ExternalInput").ap(),
        "bk_t": nc.dram_tensor("bk_t", [128, NC4], F32, kind="ExternalInput").ap(),
        "bv_row": nc.dram_tensor("bv_row", [1, C], F32, kind="ExternalInput").ap(),
        "bp_row": nc.dram_tensor("bp_row", [1, C], F32, kind="ExternalInput").ap(),
        "kaug": nc.dram_tensor("kaug", [17, 128], F32, kind="ExternalInput").ap(),
        "qaug": nc.dram_tensor("qaug", [17, 512], F32, kind="ExternalInput").ap(),
        "ones_in": nc.dram_tensor("ones_in", [128, 128], F32,
                                  kind="ExternalInput").ap(),
    }
    if general_mask:
        ins["kmask"] = nc.dram_tensor("kmask", [128, NT], F32,
                                      kind="ExternalInput").ap()
    out = nc.dram_tensor("out", [T, C], F32, kind="ExternalOutput").ap()
    with tile.TileContext(nc) as tc:
        build_attention(tc, out, ins, general_mask)
    nc.compile()
    return nc


_cached = {}


def get_program(general_mask=False):
    if general_mask not in _cached:
        _cached[general_mask] = build_program(general_mask)
    return _cached[general_mask]


def kernel(x, mask, Wq, bq, Wk, bk, Wv, bv, Wp, bp):
    per_core, general_mask = make_host_inputs(
        x, mask, Wq, bq, Wk, bk, Wv, bv, Wp, bp)
    nc = get_program(general_mask)
    res = run_bass_kernel_spmd(nc, per_core, core_ids=list(range(B)))
    out = np.stack([res.results[b]["out"] for b in range(B)], axis=0)
    return out.astype(np.float32)

